# revision 38
# baseline (speedup 1.0000x reference)
"""Distributed Trainium2 kernel for the 3-branch masked attention problem.

Sharding: 8 cores; core c handles batch b = c//2 and heads h0 = 4*(c%2) .. +4
(data + head parallel).  Each core computes QKV for its heads, the three
branch softmaxes and AV locally, then a pair-wise AllGather of the [256, 2048]
attention output (transposed) lets both cores of a batch apply the output
projection.  Host-side work is limited to sharding/layout (transposes, bf16
conversion, constant folding of d**-0.5 and the 1/3 branch average).
"""

import numpy as np
import ml_dtypes

BF16 = ml_dtypes.bfloat16

H = 8
DA, DP, DK = 2048, 1024, 1024
B, N = 4, 2048
DOUT = 512
H_LOC = 4           # heads per core
DA_H, DP_H, DK_H = DA // H, DP // H, DK // H      # 256, 128, 128
da, dp, dk = DA_H // H, DP_H // H, DK_H // H      # 32, 16, 16
DV = da + dp + dk                                 # 64
NCORES = 8

IB = 512            # query block (moving dim of dots / AV)
JB = 128            # key chunk (contract chunk of AV, M of dots)
NI = N // IB        # 4
NJ = N // JB        # 16

_CACHE = {}


def _build():
    import concourse.bass as bass
    import concourse.mybir as mybir
    import concourse.tile as tile
    from concourse import bacc
    from concourse.masks import make_identity
    from concourse.tile import add_dep_helper

    f32 = mybir.dt.float32
    bf16 = mybir.dt.bfloat16
    Exp = mybir.ActivationFunctionType.Exp
    mult = mybir.AluOpType.mult
    add = mybir.AluOpType.add

    nc = bacc.Bacc("TRN2", target_bir_lowering=False, debug=False,
                   enable_asserts=False, num_devices=NCORES)

    xT = nc.dram_tensor("xT", [DA + DP + DK, N], bf16, kind="ExternalInput")
    maskT = nc.dram_tensor("maskT", [N, N], bf16, kind="ExternalInput")
    waT = nc.dram_tensor("waT", [DA, 384], bf16, kind="ExternalInput")
    wpT = nc.dram_tensor("wpT", [DP, 384], bf16, kind="ExternalInput")
    wkT = nc.dram_tensor("wkT", [DK, 384], bf16, kind="ExternalInput")
    woutT = nc.dram_tensor("woutT", [DOUT, DOUT], bf16, kind="ExternalInput")
    bout = nc.dram_tensor("bout", [DOUT, 1], f32, kind="ExternalInput")
    out = nc.dram_tensor("out", [DOUT, N], f32, kind="ExternalOutput")

    with tile.TileContext(nc) as tc:
        with (
            tc.tile_pool(name="const", bufs=1) as cpool,
            tc.tile_pool(name="dram", bufs=1, space="DRAM") as dpool,
        ):
            # ---- constants ----
            ident_bf = cpool.tile([128, 128], bf16)
            make_identity(nc, ident_bf)
            ident_f32 = cpool.tile([128, 128], f32)
            make_identity(nc, ident_f32)

            bias_sb = cpool.tile([128, 4], f32)
            for t in range(4):
                nc.sync.dma_start(bias_sb[:, t:t + 1], bout[128 * t:128 * (t + 1), :])

            wa_sb = [cpool.tile([128, 384], bf16, name=f"wa{f}") for f in range(16)]
            for f in range(16):
                nc.sync.dma_start(wa_sb[f][:], waT[128 * f:128 * (f + 1), :])
            wp_sb = [cpool.tile([128, 384], bf16, name=f"wp{f}") for f in range(8)]
            wk_sb = [cpool.tile([128, 384], bf16, name=f"wk{f}") for f in range(8)]
            for f in range(8):
                nc.sync.dma_start(wp_sb[f][:], wpT[128 * f:128 * (f + 1), :])
                nc.sync.dma_start(wk_sb[f][:], wkT[128 * f:128 * (f + 1), :])
            wo_sb = [cpool.tile([128, DOUT], bf16, name=f"wo{f}") for f in range(4)]
            for f in range(4):
                nc.sync.dma_start(wo_sb[f][:], woutT[128 * f:128 * (f + 1), :])

            # ---- persistent activations ----
            # qT/kT per branch: [128, N]; heads live at 32-aligned partition bases
            qTa = cpool.tile([128, N], bf16)
            kTa = cpool.tile([128, N], bf16)
            qTp = cpool.tile([128, N], bf16)
            kTp = cpool.tile([128, N], bf16)
            qTk = cpool.tile([128, N], bf16)
            kTk = cpool.tile([128, N], bf16)
            # V^T combined: head h at rows 64*(h%2)+[va(32)|vp(16)|vk(16)] of tile h//2
            comb = [cpool.tile([128, N], bf16, name=f"comb{i}") for i in range(2)]
            # V_aug per head: 16 chunks of [128, 128] side by side: cols
            # 0:64 = v, col 64 = ones, 65:128 = zeros (M=128 keeps the PE
            # array fully configured so HAM doesn't throttle the clock)
            vaug = [cpool.tile([128, 128 * NJ], bf16, name=f"vaug{h}") for h in range(H_LOC)]
            # normalized attention output accumulator, [token, dv] layout
            oacc = [[cpool.tile([128, DV], f32, name=f"oacc{h}_{s}") for s in range(N // 128)]
                    for h in range(H_LOC)]
            # final transposed attention output (this core's heads)
            otc = [cpool.tile([128, N], bf16, name=f"otc{i}") for i in range(2)]

            cc_in = dpool.tile([2 * 128, N], bf16)
            cc_out = dpool.tile([4 * 128, N], bf16)

            for h in range(H_LOC):
                nc.gpsimd.memset(vaug[h][:], 0.0)
                for j in range(NJ):
                    nc.gpsimd.memset(vaug[h][:, 128 * j + 64:128 * j + 65], 1.0)

            _mctx = tc.tile_pool(name="mask", bufs=1)
            mpool = _mctx.__enter__()
            m_sb = [mpool.tile([128, N], bf16, name=f"m{j}") for j in range(NJ)]

            # =================== QKV projection ===================
            with (
                tc.tile_pool(name="xs", bufs=8) as xpool,
                tc.tile_pool(name="qkv_ps", bufs=5, space="PSUM") as qkv_ps,
                tc.tile_pool(name="vtr_ps", bufs=2, space="PSUM") as vtr_ps,
            ):
                # branch spec: (x row offset, n f-chunks, weights)
                # p and k share a v accumulator: vp at psum rows 32h+0:16,
                # vk at 32h+16:32 (host-padded weight columns) so every
                # PSUM read is 32-partition aligned.
                branches = [
                    (0, 16, wa_sb),
                    (DA, 8, wp_sb),
                    (DA + DP, 8, wk_sb),
                ]
                # pass 1: all v projections (attention needs the full
                # concatenated V, so finish it first).  Token-pair x tiles
                # ([128, 1024]) halve the DMA count.
                for tp2 in range(2):
                    t0 = 2 * IB * tp2
                    ps_va = [qkv_ps.tile([128, IB], f32, tag="qkv", name=f"psva{u}")
                             for u in range(2)]
                    ps_vpk = [qkv_ps.tile([128, IB], f32, tag="qkv", name=f"psvpk{u}")
                              for u in range(2)]
                    for bi, (fofs, nf, wsb) in enumerate(branches):
                        ps_v = ps_va if bi == 0 else ps_vpk
                        for f in range(nf):
                            xt = xpool.tile([128, 2 * IB], bf16, tag="x")
                            nc.sync.dma_start(
                                xt[:], xT[fofs + 128 * f:fofs + 128 * (f + 1),
                                          t0:t0 + 2 * IB])
                            vst = (f == 0) and bi != 2
                            vsp = (f == nf - 1) and bi != 1
                            for u in range(2):
                                nc.tensor.matmul(ps_v[u][:], wsb[f][:, 256:384],
                                                 xt[:, IB * u:IB * (u + 1)],
                                                 start=vst, stop=vsp)
                    for u in range(2):
                        tsl = slice(t0 + IB * u, t0 + IB * (u + 1))
                        for h in range(H_LOC):
                            nc.vector.tensor_copy(
                                comb[h // 2][64 * (h % 2):64 * (h % 2) + da, tsl],
                                ps_va[u][da * h:da * (h + 1), :])
                            nc.vector.tensor_copy(
                                comb[h // 2][64 * (h % 2) + da:64 * (h % 2) + 64, tsl],
                                ps_vpk[u][32 * h:32 * (h + 1), :])

                # V_aug: transpose comb chunks
                for j in range(NJ):
                    jsl = slice(128 * j, 128 * (j + 1))
                    for c in range(2):
                        tp = vtr_ps.tile([128, 128], bf16, tag="vtr")
                        nc.tensor.transpose(tp[:], comb[c][:, jsl], ident_bf[:])
                        nc.vector.tensor_copy(vaug[2 * c][:, 128 * j:128 * j + 64], tp[:, 0:64])
                        nc.vector.tensor_copy(vaug[2 * c + 1][:, 128 * j:128 * j + 64], tp[:, 64:128])

                # pass 2: q/k per branch — branch a first so its attention
                # can start while p/k still project
                for bi, (fofs, nf, wsb) in enumerate(branches):
                    if bi == 1:
                        # mask tiles: issued once branch-a q/k DMAs are in the
                        # queues; they land before attention consumes them
                        for j in range(NJ):
                            nc.sync.dma_start(m_sb[j][:],
                                              maskT[128 * j:128 * (j + 1), :])
                    for tp2 in range(2):
                        t0 = 2 * IB * tp2
                        ps_q = [qkv_ps.tile([128, IB], f32, tag="qkv", name=f"psq{u}")
                                for u in range(2)]
                        ps_k = [qkv_ps.tile([128, IB], f32, tag="qkv", name=f"psk{u}")
                                for u in range(2)]
                        for f in range(nf):
                            xt = xpool.tile([128, 2 * IB], bf16, tag="x")
                            nc.sync.dma_start(
                                xt[:], xT[fofs + 128 * f:fofs + 128 * (f + 1),
                                          t0:t0 + 2 * IB])
                            st, sp = (f == 0), (f == nf - 1)
                            w = wsb[f]
                            for u in range(2):
                                nc.tensor.matmul(ps_q[u][:], w[:, 0:128],
                                                 xt[:, IB * u:IB * (u + 1)],
                                                 start=st, stop=sp)
                                nc.tensor.matmul(ps_k[u][:], w[:, 128:256],
                                                 xt[:, IB * u:IB * (u + 1)],
                                                 start=st, stop=sp)
                        qT_t = (qTa, qTp, qTk)[bi]
                        kT_t = (kTa, kTp, kTk)[bi]
                        for u in range(2):
                            tsl = slice(t0 + IB * u, t0 + IB * (u + 1))
                            nc.vector.tensor_copy(qT_t[:, tsl], ps_q[u][:])
                            nc.vector.tensor_copy(kT_t[:, tsl], ps_k[u][:])

            # =================== attention ===================
            with (
                tc.tile_pool(name="s_ps", bufs=2, space="PSUM") as s_ps_pool,
                tc.tile_pool(name="o_ps", bufs=4, space="PSUM") as o_ps_pool,
                tc.tile_pool(name="ep", bufs=6) as epool,
                tc.tile_pool(name="pp", bufs=6) as ppool,
                tc.tile_pool(name="ob", bufs=3) as opool,
                tc.tile_pool(name="rr", bufs=4) as rpool,
            ):
                battn = [(qTa, kTa, da), (qTp, kTp, dp), (qTk, kTk, dk)]
                for bi, (qT_t, kT_t, d) in enumerate(battn):
                    for I in range(NI):
                        isl = slice(IB * I, IB * (I + 1))
                        # 2 halves of 2 heads each: each half has its own
                        # 2-bank S tile, so dots of one half overlap exp
                        # of the other; within a half the 2 row-disjoint
                        # dots are chained adjacent to run concurrently
                        o_ps_h = [o_ps_pool.tile([128, IB], f32, tag="o",
                                                 name=f"ops{h}")
                                  for h in range(H_LOC)]

                        def emit_av(jj, hf, pp_sb):
                            for hh in range(2):
                                h = 2 * hf + hh
                                nc.tensor.matmul(
                                    o_ps_h[h][:],
                                    vaug[h][:, 128 * jj:128 * (jj + 1)],
                                    pp_sb[:, IB * hh:IB * (hh + 1)],
                                    start=(jj == 0), stop=(jj == NJ - 1),
                                    skip_group_check=True)

                        # while DVE digests the previous block's epilogue
                        # (first ~5 j's), mask-multiplies go to gpsimd and
                        # their AVs are emitted two j's later so the slow
                        # gpsimd op can't head-of-line-block the PE stream
                        first_blk = (bi == 0 and I == 0)
                        av_backlog = []
                        for j in range(NJ):
                            for half in range(2):
                                s_ps = s_ps_pool.tile([128, 2 * IB], f32,
                                                      tag="s", name=f"sh{half}")
                                dots = []
                                for hh in range(2):
                                    h = 2 * half + hh
                                    pb = 32 * h
                                    mm = nc.tensor.matmul(
                                        s_ps[:, IB * hh:IB * (hh + 1)],
                                        kT_t[pb:pb + d, 128 * j:128 * (j + 1)],
                                        qT_t[pb:pb + d, isl],
                                        start=True, stop=True,
                                        tile_position=(pb, 0))
                                    if dots:
                                        add_dep_helper(mm.ins, dots[-1].ins,
                                                       sync=False,
                                                       reason="chain dots")
                                    dots.append(mm)
                                if half == 0:
                                    while av_backlog and av_backlog[0][0] <= j:
                                        _, jj, hf, pp_sb = av_backlog.pop(0)
                                        emit_av(jj, hf, pp_sb)
                                e_sb = epool.tile([128, 2 * IB], bf16, tag="e")
                                nc.scalar.activation(e_sb[:], s_ps[:], Exp)
                                p_sb = ppool.tile([128, 2 * IB], bf16, tag="p")
                                m_bc = m_sb[j][:, None, isl].broadcast_to(
                                    [128, 2, IB])
                                on_gp = j < 5 and not first_blk
                                teng = nc.gpsimd if on_gp else nc.vector
                                teng.tensor_tensor(
                                    p_sb[:].rearrange("p (g i) -> p g i", g=2),
                                    e_sb[:].rearrange("p (g i) -> p g i", g=2),
                                    m_bc, op=mult)
                                if on_gp:
                                    av_backlog.append((j + 2, j, half, p_sb))
                                else:
                                    emit_av(j, half, p_sb)
                        for _, jj, hf, pp_sb in av_backlog:
                            emit_av(jj, hf, pp_sb)
                        # epilogue: drain all four accumulators first so the
                        # next block's AVs get PSUM slots immediately, then
                        # normalize + accumulate
                        o_sbs = []
                        for h in range(H_LOC):
                            o_sb = opool.tile([65, IB], f32, tag="osb",
                                              name=f"osb{h}")
                            nc.vector.tensor_copy(o_sb[:], o_ps_h[h][0:65, :])
                            o_sbs.append(o_sb)
                        for h in range(H_LOC):
                            for s in range(IB // 128):
                                tp = o_ps_pool.tile([128, 65], f32, tag="o",
                                                    name="tps")
                                nc.tensor.transpose(
                                    tp[:], o_sbs[h][:, 128 * s:128 * (s + 1)],
                                    ident_f32[0:65, 0:65])
                                r_sb = rpool.tile([128, 1], f32, tag="r")
                                nc.vector.reciprocal(r_sb[:], tp[:, 64:65])
                                at = oacc[h][4 * I + s]
                                if bi == 0:
                                    nc.vector.tensor_scalar_mul(at[:], tp[:, 0:DV], r_sb[:])
                                else:
                                    nc.vector.scalar_tensor_tensor(
                                        at[:], tp[:, 0:DV], r_sb[:], at[:],
                                        op0=mult, op1=add)
                                if bi == 2:
                                    tp2 = o_ps_pool.tile([DV, 128], f32,
                                                         tag="o", name="t2")
                                    nc.tensor.transpose(tp2[:], at[:],
                                                        ident_f32[:])
                                    sl = 4 * I + s
                                    nc.vector.tensor_copy(
                                        otc[h // 2][64 * (h % 2):64 * (h % 2) + DV,
                                                    128 * sl:128 * (sl + 1)],
                                        tp2[:])

            _mctx.__exit__(None, None, None)

            # =================== gather + output projection ===================
            with (
                tc.tile_pool(name="fin", bufs=2) as fpool,
                tc.tile_pool(name="otf", bufs=1) as otfpool,
                tc.tile_pool(name="f_ps", bufs=2, space="PSUM") as f_ps_pool,
            ):
                for c in range(2):
                    nc.sync.dma_start(cc_in[128 * c:128 * (c + 1), :], otc[c][:])
                nc.gpsimd.collective_compute(
                    "AllGather",
                    mybir.AluOpType.bypass,
                    replica_groups=[[0, 1], [2, 3], [4, 5], [6, 7]],
                    ins=[cc_in.opt()],
                    outs=[cc_out.opt()],
                )
                otf = [otfpool.tile([128, N], bf16, name=f"otf{i}") for i in range(4)]
                for c in range(4):
                    nc.sync.dma_start(otf[c][:], cc_out[128 * c:128 * (c + 1), :])
                for ot in range(4):
                    for I2 in range(4):
                        i2sl = slice(512 * I2, 512 * (I2 + 1))
                        ps = f_ps_pool.tile([128, 512], f32, tag="f")
                        for ic in range(4):
                            nc.tensor.matmul(
                                ps[:], wo_sb[ic][:, 128 * ot:128 * (ot + 1)],
                                otf[ic][:, i2sl],
                                start=(ic == 0), stop=(ic == 3))
                        fin = fpool.tile([128, 512], f32, tag="fin")
                        nc.vector.tensor_scalar_add(fin[:], ps[:], bias_sb[:, ot:ot + 1])
                        nc.sync.dma_start(out[128 * ot:128 * (ot + 1), i2sl], fin[:])

    nc.compile()
    return nc


def _prep_core(c, x, W_a, W_p, W_k, W_out, b_out, mask):
    b = c // 2
    h0 = H_LOC * (c % 2)

    xT = np.ascontiguousarray(x[b].T).astype(BF16)
    maskT = np.ascontiguousarray(mask[b, 0].T).astype(BF16)

    qa = W_a[da * h0: da * (h0 + H_LOC), :] * (DA ** -0.5)
    ka = W_a[DA_H + da * h0: DA_H + da * (h0 + H_LOC), :]
    va = W_a[2 * DA_H + da * h0: 2 * DA_H + da * (h0 + H_LOC), :]
    waT = np.concatenate([qa.T, ka.T, va.T], axis=1).astype(BF16)

    def pk_branch(W, D, D_H, d, vcol_ofs):
        qpad = np.zeros((D, 128), np.float32)
        kpad = np.zeros((D, 128), np.float32)
        vpad = np.zeros((D, 128), np.float32)
        for h in range(H_LOC):
            qpad[:, 32 * h:32 * h + d] = W[d * (h0 + h): d * (h0 + h + 1), :].T * (D ** -0.5)
            kpad[:, 32 * h:32 * h + d] = W[D_H + d * (h0 + h): D_H + d * (h0 + h + 1), :].T
            vpad[:, 32 * h + vcol_ofs:32 * h + vcol_ofs + d] = \
                W[2 * D_H + d * (h0 + h): 2 * D_H + d * (h0 + h + 1), :].T
        return np.concatenate([qpad, kpad, vpad], axis=1).astype(BF16)

    wpT = pk_branch(W_p, DP, DP_H, dp, 0)
    wkT = pk_branch(W_k, DK, DK_H, dk, 16)

    woutT = np.ascontiguousarray((W_out / 3.0).T).astype(BF16)
    bout = np.ascontiguousarray(b_out.reshape(DOUT, 1)).astype(np.float32)

    return {
        "xT": np.ascontiguousarray(xT),
        "maskT": np.ascontiguousarray(maskT),
        "waT": np.ascontiguousarray(waT),
        "wpT": np.ascontiguousarray(wpT),
        "wkT": np.ascontiguousarray(wkT),
        "woutT": woutT,
        "bout": bout,
    }


def kernel(x, W_a, W_p, W_k, W_out, b_out, mask):
    from concourse.bass_utils import run_bass_kernel_spmd

    x = np.asarray(x, np.float32)
    W_a = np.asarray(W_a, np.float32)
    W_p = np.asarray(W_p, np.float32)
    W_k = np.asarray(W_k, np.float32)
    W_out = np.asarray(W_out, np.float32)
    b_out = np.asarray(b_out, np.float32)
    mask = np.asarray(mask)

    if "nc" not in _CACHE:
        _CACHE["nc"] = _build()
    nc = _CACHE["nc"]

    in_maps = [_prep_core(c, x, W_a, W_p, W_k, W_out, b_out, mask)
               for c in range(NCORES)]
    res = run_bass_kernel_spmd(nc, in_maps, core_ids=list(range(NCORES)))

    outs = []
    for b in range(B):
        outs.append(np.asarray(res.results[2 * b]["out"], np.float32).T)
    return np.stack(outs, axis=0)


# revision 41
# speedup vs baseline: 1.0641x; 1.0641x over previous
"""Distributed Trainium2 kernel for the 3-branch masked attention problem.

Sharding: 8 cores; core c handles batch b = c//2 and heads h0 = 4*(c%2) .. +4
(data + head parallel).  Each core computes QKV for its heads, the three
branch softmaxes and AV locally, then a pair-wise AllGather of the [256, 2048]
attention output (transposed) lets both cores of a batch apply the output
projection.  Host-side work is limited to sharding/layout (transposes, bf16
conversion, constant folding of d**-0.5 and the 1/3 branch average).
"""

import numpy as np
import ml_dtypes

BF16 = ml_dtypes.bfloat16

H = 8
DA, DP, DK = 2048, 1024, 1024
B, N = 4, 2048
DOUT = 512
H_LOC = 4           # heads per core
DA_H, DP_H, DK_H = DA // H, DP // H, DK // H      # 256, 128, 128
da, dp, dk = DA_H // H, DP_H // H, DK_H // H      # 32, 16, 16
DV = da + dp + dk                                 # 64
NCORES = 8

IB = 512            # query block (moving dim of dots / AV)
JB = 128            # key chunk (contract chunk of AV, M of dots)
NI = N // IB        # 4
NJ = N // JB        # 16

_CACHE = {}


def _build():
    import concourse.bass as bass
    import concourse.mybir as mybir
    import concourse.tile as tile
    from concourse import bacc
    from concourse.masks import make_identity
    from concourse.tile import add_dep_helper

    f32 = mybir.dt.float32
    bf16 = mybir.dt.bfloat16
    Exp = mybir.ActivationFunctionType.Exp
    mult = mybir.AluOpType.mult
    add = mybir.AluOpType.add

    nc = bacc.Bacc("TRN2", target_bir_lowering=False, debug=False,
                   enable_asserts=False, num_devices=NCORES)

    xT = nc.dram_tensor("xT", [DA + DP + DK, N], bf16, kind="ExternalInput")
    maskT = nc.dram_tensor("maskT", [N, N], bf16, kind="ExternalInput")
    waT = nc.dram_tensor("waT", [DA, 384], bf16, kind="ExternalInput")
    wpT = nc.dram_tensor("wpT", [DP, 384], bf16, kind="ExternalInput")
    wkT = nc.dram_tensor("wkT", [DK, 384], bf16, kind="ExternalInput")
    woutT = nc.dram_tensor("woutT", [DOUT, DOUT], bf16, kind="ExternalInput")
    bout = nc.dram_tensor("bout", [DOUT, 1], f32, kind="ExternalInput")
    out = nc.dram_tensor("out", [DOUT, N], f32, kind="ExternalOutput")

    with tile.TileContext(nc) as tc:
        with (
            tc.tile_pool(name="const", bufs=1) as cpool,
            tc.tile_pool(name="dram", bufs=1, space="DRAM") as dpool,
        ):
            # ---- constants ----
            ident_bf = cpool.tile([128, 128], bf16)
            make_identity(nc, ident_bf)
            ident_f32 = cpool.tile([128, 128], f32)
            make_identity(nc, ident_f32)

            bias_sb = cpool.tile([128, 4], f32)
            for t in range(4):
                nc.sync.dma_start(bias_sb[:, t:t + 1], bout[128 * t:128 * (t + 1), :])

            wa_sb = [cpool.tile([128, 384], bf16, name=f"wa{f}") for f in range(16)]
            for f in range(16):
                nc.sync.dma_start(wa_sb[f][:], waT[128 * f:128 * (f + 1), :])
            wp_sb = [cpool.tile([128, 384], bf16, name=f"wp{f}") for f in range(8)]
            wk_sb = [cpool.tile([128, 384], bf16, name=f"wk{f}") for f in range(8)]
            for f in range(8):
                nc.sync.dma_start(wp_sb[f][:], wpT[128 * f:128 * (f + 1), :])
                nc.sync.dma_start(wk_sb[f][:], wkT[128 * f:128 * (f + 1), :])
            wo_sb = [cpool.tile([128, DOUT], bf16, name=f"wo{f}") for f in range(4)]
            for f in range(4):
                nc.sync.dma_start(wo_sb[f][:], woutT[128 * f:128 * (f + 1), :])

            # ---- persistent activations ----
            # qT/kT per branch: [128, N]; heads live at 32-aligned partition bases
            qTa = cpool.tile([128, N], bf16)
            kTa = cpool.tile([128, N], bf16)
            qTp = cpool.tile([128, N], bf16)
            kTp = cpool.tile([128, N], bf16)
            qTk = cpool.tile([128, N], bf16)
            kTk = cpool.tile([128, N], bf16)
            # V^T combined: head h at rows 64*(h%2)+[va(32)|vp(16)|vk(16)] of tile h//2
            comb = [cpool.tile([128, N], bf16, name=f"comb{i}") for i in range(2)]
            # V_aug per head: 16 chunks of [128, 128] side by side: cols
            # 0:64 = v, col 64 = ones, 65:128 = zeros (M=128 keeps the PE
            # array fully configured so HAM doesn't throttle the clock)
            vaug = [cpool.tile([128, 128 * NJ], bf16, name=f"vaug{h}") for h in range(H_LOC)]
            # normalized attention output accumulator, [token, dv] layout;
            # one [128, 16*64] tile per head (token slice sl at cols 64*sl)
            oacc = [cpool.tile([128, (N // 128) * DV], f32, name=f"oacc{h}")
                    for h in range(H_LOC)]
            # final transposed attention output (this core's heads)
            otc = [cpool.tile([128, N], bf16, name=f"otc{i}") for i in range(2)]

            cc_in = dpool.tile([2 * 128, N], bf16)
            cc_out = dpool.tile([4 * 128, N], bf16)

            for h in range(H_LOC):
                nc.gpsimd.memset(vaug[h][:], 0.0)
                for j in range(NJ):
                    nc.gpsimd.memset(vaug[h][:, 128 * j + 64:128 * j + 65], 1.0)

            _mctx = tc.tile_pool(name="mask", bufs=1)
            mpool = _mctx.__enter__()
            m_sb = [mpool.tile([128, N], bf16, name=f"m{j}") for j in range(NJ)]

            # =================== QKV projection ===================
            with (
                tc.tile_pool(name="xs", bufs=8) as xpool,
                tc.tile_pool(name="qkv_ps", bufs=5, space="PSUM") as qkv_ps,
                tc.tile_pool(name="vtr_ps", bufs=2, space="PSUM") as vtr_ps,
            ):
                # branch spec: (x row offset, n f-chunks, weights)
                # p and k share a v accumulator: vp at psum rows 32h+0:16,
                # vk at 32h+16:32 (host-padded weight columns) so every
                # PSUM read is 32-partition aligned.
                branches = [
                    (0, 16, wa_sb),
                    (DA, 8, wp_sb),
                    (DA + DP, 8, wk_sb),
                ]
                # pass 1: all v projections (attention needs the full
                # concatenated V, so finish it first).  Token-pair x tiles
                # ([128, 1024]) halve the DMA count.
                for tp2 in range(2):
                    t0 = 2 * IB * tp2
                    ps_va = [qkv_ps.tile([128, IB], f32, tag="qkv", name=f"psva{u}")
                             for u in range(2)]
                    ps_vpk = [qkv_ps.tile([128, IB], f32, tag="qkv", name=f"psvpk{u}")
                              for u in range(2)]
                    for bi, (fofs, nf, wsb) in enumerate(branches):
                        ps_v = ps_va if bi == 0 else ps_vpk
                        for f in range(nf):
                            xt = xpool.tile([128, 2 * IB], bf16, tag="x")
                            nc.sync.dma_start(
                                xt[:], xT[fofs + 128 * f:fofs + 128 * (f + 1),
                                          t0:t0 + 2 * IB])
                            vst = (f == 0) and bi != 2
                            vsp = (f == nf - 1) and bi != 1
                            for u in range(2):
                                nc.tensor.matmul(ps_v[u][:], wsb[f][:, 256:384],
                                                 xt[:, IB * u:IB * (u + 1)],
                                                 start=vst, stop=vsp)
                    for u in range(2):
                        tsl = slice(t0 + IB * u, t0 + IB * (u + 1))
                        for h in range(H_LOC):
                            nc.vector.tensor_copy(
                                comb[h // 2][64 * (h % 2):64 * (h % 2) + da, tsl],
                                ps_va[u][da * h:da * (h + 1), :])
                            nc.vector.tensor_copy(
                                comb[h // 2][64 * (h % 2) + da:64 * (h % 2) + 64, tsl],
                                ps_vpk[u][32 * h:32 * (h + 1), :])

                # V_aug: transpose comb chunks
                for j in range(NJ):
                    jsl = slice(128 * j, 128 * (j + 1))
                    for c in range(2):
                        tp = vtr_ps.tile([128, 128], bf16, tag="vtr")
                        nc.tensor.transpose(tp[:], comb[c][:, jsl], ident_bf[:])
                        nc.vector.tensor_copy(vaug[2 * c][:, 128 * j:128 * j + 64], tp[:, 0:64])
                        nc.vector.tensor_copy(vaug[2 * c + 1][:, 128 * j:128 * j + 64], tp[:, 64:128])

                # pass 2: q/k per branch — branch a first so its attention
                # can start while p/k still project
                for bi, (fofs, nf, wsb) in enumerate(branches):
                    if bi == 1:
                        # mask tiles: issued once branch-a q/k DMAs are in the
                        # queues; they land before attention consumes them
                        for j in range(NJ):
                            nc.sync.dma_start(m_sb[j][:],
                                              maskT[128 * j:128 * (j + 1), :])
                    for tp2 in range(2):
                        t0 = 2 * IB * tp2
                        ps_q = [qkv_ps.tile([128, IB], f32, tag="qkv", name=f"psq{u}")
                                for u in range(2)]
                        ps_k = [qkv_ps.tile([128, IB], f32, tag="qkv", name=f"psk{u}")
                                for u in range(2)]
                        for f in range(nf):
                            xt = xpool.tile([128, 2 * IB], bf16, tag="x")
                            nc.sync.dma_start(
                                xt[:], xT[fofs + 128 * f:fofs + 128 * (f + 1),
                                          t0:t0 + 2 * IB])
                            st, sp = (f == 0), (f == nf - 1)
                            w = wsb[f]
                            for u in range(2):
                                nc.tensor.matmul(ps_q[u][:], w[:, 0:128],
                                                 xt[:, IB * u:IB * (u + 1)],
                                                 start=st, stop=sp)
                                nc.tensor.matmul(ps_k[u][:], w[:, 128:256],
                                                 xt[:, IB * u:IB * (u + 1)],
                                                 start=st, stop=sp)
                        qT_t = (qTa, qTp, qTk)[bi]
                        kT_t = (kTa, kTp, kTk)[bi]
                        for u in range(2):
                            tsl = slice(t0 + IB * u, t0 + IB * (u + 1))
                            nc.vector.tensor_copy(qT_t[:, tsl], ps_q[u][:])
                            nc.vector.tensor_copy(kT_t[:, tsl], ps_k[u][:])

            # =================== attention ===================
            with (
                tc.tile_pool(name="s_ps", bufs=2, space="PSUM") as s_ps_pool,
                tc.tile_pool(name="o_ps", bufs=4, space="PSUM") as o_ps_pool,
                tc.tile_pool(name="ep", bufs=6) as epool,
                tc.tile_pool(name="pp", bufs=6) as ppool,
                tc.tile_pool(name="ob", bufs=3) as opool,
                tc.tile_pool(name="rr", bufs=4) as rpool,
            ):
                battn = [(qTa, kTa, da), (qTp, kTp, dp), (qTk, kTk, dk)]
                for bi, (qT_t, kT_t, d) in enumerate(battn):
                    for I in range(NI):
                        isl = slice(IB * I, IB * (I + 1))
                        # 2 halves of 2 heads each: each half has its own
                        # 2-bank S tile, so dots of one half overlap exp
                        # of the other; within a half the 2 row-disjoint
                        # dots are chained adjacent to run concurrently
                        o_ps_h = [o_ps_pool.tile([128, IB], f32, tag="o",
                                                 name=f"ops{h}")
                                  for h in range(H_LOC)]

                        def emit_av(jj, hf, pp_sb):
                            for hh in range(2):
                                h = 2 * hf + hh
                                nc.tensor.matmul(
                                    o_ps_h[h][:],
                                    vaug[h][:, 128 * jj:128 * (jj + 1)],
                                    pp_sb[:, IB * hh:IB * (hh + 1)],
                                    start=(jj == 0), stop=(jj == NJ - 1),
                                    skip_group_check=True)

                        # while DVE digests the previous block's epilogue
                        # (first ~5 j's), mask-multiplies go to gpsimd and
                        # their AVs are emitted two j's later so the slow
                        # gpsimd op can't head-of-line-block the PE stream
                        first_blk = (bi == 0 and I == 0)
                        av_backlog = []
                        for j in range(NJ):
                            for half in range(2):
                                s_ps = s_ps_pool.tile([128, 2 * IB], f32,
                                                      tag="s", name=f"sh{half}")
                                dots = []
                                for hh in range(2):
                                    h = 2 * half + hh
                                    pb = 32 * h
                                    mm = nc.tensor.matmul(
                                        s_ps[:, IB * hh:IB * (hh + 1)],
                                        kT_t[pb:pb + d, 128 * j:128 * (j + 1)],
                                        qT_t[pb:pb + d, isl],
                                        start=True, stop=True,
                                        tile_position=(pb, 0))
                                    if dots:
                                        add_dep_helper(mm.ins, dots[-1].ins,
                                                       sync=False,
                                                       reason="chain dots")
                                    dots.append(mm)
                                if half == 0:
                                    while av_backlog and av_backlog[0][0] <= j:
                                        _, jj, hf, pp_sb = av_backlog.pop(0)
                                        emit_av(jj, hf, pp_sb)
                                e_sb = epool.tile([128, 2 * IB], bf16, tag="e")
                                nc.scalar.activation(e_sb[:], s_ps[:], Exp)
                                p_sb = ppool.tile([128, 2 * IB], bf16, tag="p")
                                m_bc = m_sb[j][:, None, isl].broadcast_to(
                                    [128, 2, IB])
                                on_gp = False
                                teng = nc.vector
                                teng.tensor_tensor(
                                    p_sb[:].rearrange("p (g i) -> p g i", g=2),
                                    e_sb[:].rearrange("p (g i) -> p g i", g=2),
                                    m_bc, op=mult)
                                if on_gp:
                                    av_backlog.append((j + 2, j, half, p_sb))
                                else:
                                    emit_av(j, half, p_sb)
                        for _, jj, hf, pp_sb in av_backlog:
                            emit_av(jj, hf, pp_sb)
                        # epilogue: drain all four accumulators first so the
                        # next block's AVs get PSUM slots immediately, then
                        # normalize + accumulate
                        o_sbs = []
                        for h in range(H_LOC):
                            o_sb = opool.tile([65, IB], f32, tag="osb",
                                              name=f"osb{h}")
                            nc.vector.tensor_copy(o_sb[:], o_ps_h[h][0:65, :])
                            o_sbs.append(o_sb)
                        for h in range(H_LOC):
                            # all 4 token-slices of this head transpose into
                            # one PSUM tile; one batched reciprocal and two
                            # wide tensor ops do the normalize + accumulate
                            tp = o_ps_pool.tile([128, 4 * 65], f32, tag="o",
                                                name="tp4")
                            for s in range(IB // 128):
                                nc.tensor.transpose(
                                    tp[:, 65 * s:65 * (s + 1)],
                                    o_sbs[h][:, 128 * s:128 * (s + 1)],
                                    ident_f32[0:65, 0:65])
                            tp3 = tp[:].rearrange("p (s c) -> p s c", c=65)
                            r4 = rpool.tile([128, 4], f32, tag="r")
                            nc.vector.reciprocal(
                                r4[:].rearrange("p (s c) -> p s c", c=1),
                                tp3[:, :, 64:65])
                            num = tp3[:, :, 0:DV]
                            rbc = r4[:][:, :, None].broadcast_to([128, 4, DV])
                            at = oacc[h][:, 4 * DV * I:4 * DV * (I + 1)] \
                                .rearrange("p (s c) -> p s c", c=DV)
                            if bi == 0:
                                nc.vector.tensor_tensor(at, num, rbc, op=mult)
                            else:
                                tmp = opool.tile([128, 4 * DV], f32, tag="tmp")
                                nc.vector.tensor_tensor(
                                    tmp[:].rearrange("p (s c) -> p s c", c=DV),
                                    num, rbc, op=mult)
                                nc.vector.tensor_tensor(at, at,
                                                        tmp[:].rearrange(
                                                            "p (s c) -> p s c",
                                                            c=DV),
                                                        op=add)
                            if bi == 2:
                                for s in range(IB // 128):
                                    sl = 4 * I + s
                                    tp2 = o_ps_pool.tile([DV, 128], f32,
                                                         tag="o", name="t2")
                                    nc.tensor.transpose(
                                        tp2[:],
                                        oacc[h][:, DV * sl:DV * (sl + 1)],
                                        ident_f32[:])
                                    nc.vector.tensor_copy(
                                        otc[h // 2][64 * (h % 2):64 * (h % 2) + DV,
                                                    128 * sl:128 * (sl + 1)],
                                        tp2[:])

            _mctx.__exit__(None, None, None)

            # =================== gather + output projection ===================
            with (
                tc.tile_pool(name="fin", bufs=2) as fpool,
                tc.tile_pool(name="otf", bufs=1) as otfpool,
                tc.tile_pool(name="f_ps", bufs=2, space="PSUM") as f_ps_pool,
            ):
                for c in range(2):
                    nc.sync.dma_start(cc_in[128 * c:128 * (c + 1), :], otc[c][:])
                nc.gpsimd.collective_compute(
                    "AllGather",
                    mybir.AluOpType.bypass,
                    replica_groups=[[0, 1], [2, 3], [4, 5], [6, 7]],
                    ins=[cc_in.opt()],
                    outs=[cc_out.opt()],
                )
                otf = [otfpool.tile([128, N], bf16, name=f"otf{i}") for i in range(4)]
                for c in range(4):
                    nc.sync.dma_start(otf[c][:], cc_out[128 * c:128 * (c + 1), :])
                for ot in range(4):
                    for I2 in range(4):
                        i2sl = slice(512 * I2, 512 * (I2 + 1))
                        ps = f_ps_pool.tile([128, 512], f32, tag="f")
                        for ic in range(4):
                            nc.tensor.matmul(
                                ps[:], wo_sb[ic][:, 128 * ot:128 * (ot + 1)],
                                otf[ic][:, i2sl],
                                start=(ic == 0), stop=(ic == 3))
                        fin = fpool.tile([128, 512], f32, tag="fin")
                        nc.vector.tensor_scalar_add(fin[:], ps[:], bias_sb[:, ot:ot + 1])
                        nc.sync.dma_start(out[128 * ot:128 * (ot + 1), i2sl], fin[:])

    nc.compile()
    return nc


def _prep_core(c, x, W_a, W_p, W_k, W_out, b_out, mask):
    b = c // 2
    h0 = H_LOC * (c % 2)

    xT = np.ascontiguousarray(x[b].T).astype(BF16)
    maskT = np.ascontiguousarray(mask[b, 0].T).astype(BF16)

    qa = W_a[da * h0: da * (h0 + H_LOC), :] * (DA ** -0.5)
    ka = W_a[DA_H + da * h0: DA_H + da * (h0 + H_LOC), :]
    va = W_a[2 * DA_H + da * h0: 2 * DA_H + da * (h0 + H_LOC), :]
    waT = np.concatenate([qa.T, ka.T, va.T], axis=1).astype(BF16)

    def pk_branch(W, D, D_H, d, vcol_ofs):
        qpad = np.zeros((D, 128), np.float32)
        kpad = np.zeros((D, 128), np.float32)
        vpad = np.zeros((D, 128), np.float32)
        for h in range(H_LOC):
            qpad[:, 32 * h:32 * h + d] = W[d * (h0 + h): d * (h0 + h + 1), :].T * (D ** -0.5)
            kpad[:, 32 * h:32 * h + d] = W[D_H + d * (h0 + h): D_H + d * (h0 + h + 1), :].T
            vpad[:, 32 * h + vcol_ofs:32 * h + vcol_ofs + d] = \
                W[2 * D_H + d * (h0 + h): 2 * D_H + d * (h0 + h + 1), :].T
        return np.concatenate([qpad, kpad, vpad], axis=1).astype(BF16)

    wpT = pk_branch(W_p, DP, DP_H, dp, 0)
    wkT = pk_branch(W_k, DK, DK_H, dk, 16)

    woutT = np.ascontiguousarray((W_out / 3.0).T).astype(BF16)
    bout = np.ascontiguousarray(b_out.reshape(DOUT, 1)).astype(np.float32)

    return {
        "xT": np.ascontiguousarray(xT),
        "maskT": np.ascontiguousarray(maskT),
        "waT": np.ascontiguousarray(waT),
        "wpT": np.ascontiguousarray(wpT),
        "wkT": np.ascontiguousarray(wkT),
        "woutT": woutT,
        "bout": bout,
    }


def kernel(x, W_a, W_p, W_k, W_out, b_out, mask):
    from concourse.bass_utils import run_bass_kernel_spmd

    x = np.asarray(x, np.float32)
    W_a = np.asarray(W_a, np.float32)
    W_p = np.asarray(W_p, np.float32)
    W_k = np.asarray(W_k, np.float32)
    W_out = np.asarray(W_out, np.float32)
    b_out = np.asarray(b_out, np.float32)
    mask = np.asarray(mask)

    if "nc" not in _CACHE:
        _CACHE["nc"] = _build()
    nc = _CACHE["nc"]

    in_maps = [_prep_core(c, x, W_a, W_p, W_k, W_out, b_out, mask)
               for c in range(NCORES)]
    res = run_bass_kernel_spmd(nc, in_maps, core_ids=list(range(NCORES)))

    outs = []
    for b in range(B):
        outs.append(np.asarray(res.results[2 * b]["out"], np.float32).T)
    return np.stack(outs, axis=0)


# revision 43
# speedup vs baseline: 1.1459x; 1.0769x over previous
"""Distributed Trainium2 kernel for the 3-branch masked attention problem.

Sharding: 8 cores; core c handles batch b = c//2 and heads h0 = 4*(c%2) .. +4
(data + head parallel).  Each core computes QKV for its heads, the three
branch softmaxes and AV locally, then a pair-wise AllGather of the [256, 2048]
attention output (transposed) lets both cores of a batch apply the output
projection.  Host-side work is limited to sharding/layout (transposes, bf16
conversion, constant folding of d**-0.5 and the 1/3 branch average).
"""

import numpy as np
import ml_dtypes

BF16 = ml_dtypes.bfloat16

H = 8
DA, DP, DK = 2048, 1024, 1024
B, N = 4, 2048
DOUT = 512
H_LOC = 4           # heads per core
DA_H, DP_H, DK_H = DA // H, DP // H, DK // H      # 256, 128, 128
da, dp, dk = DA_H // H, DP_H // H, DK_H // H      # 32, 16, 16
DV = da + dp + dk                                 # 64
NCORES = 8

IB = 512            # query block (moving dim of dots / AV)
JB = 128            # key chunk (contract chunk of AV, M of dots)
NI = N // IB        # 4
NJ = N // JB        # 16

_CACHE = {}


def _build():
    import concourse.bass as bass
    import concourse.mybir as mybir
    import concourse.tile as tile
    from concourse import bacc
    from concourse.masks import make_identity
    from concourse.tile import add_dep_helper

    f32 = mybir.dt.float32
    bf16 = mybir.dt.bfloat16
    Exp = mybir.ActivationFunctionType.Exp
    mult = mybir.AluOpType.mult
    add = mybir.AluOpType.add

    nc = bacc.Bacc("TRN2", target_bir_lowering=False, debug=False,
                   enable_asserts=False, num_devices=NCORES)

    xT = nc.dram_tensor("xT", [DA + DP + DK, N], bf16, kind="ExternalInput")
    maskT = nc.dram_tensor("maskT", [N, N], bf16, kind="ExternalInput")
    waT = nc.dram_tensor("waT", [DA, 384], bf16, kind="ExternalInput")
    wpT = nc.dram_tensor("wpT", [DP, 384], bf16, kind="ExternalInput")
    wkT = nc.dram_tensor("wkT", [DK, 384], bf16, kind="ExternalInput")
    woutT = nc.dram_tensor("woutT", [DOUT, DOUT], bf16, kind="ExternalInput")
    bout = nc.dram_tensor("bout", [DOUT, 1], f32, kind="ExternalInput")
    out = nc.dram_tensor("out", [DOUT, N], f32, kind="ExternalOutput")

    with tile.TileContext(nc) as tc:
        with (
            tc.tile_pool(name="const", bufs=1) as cpool,
            tc.tile_pool(name="dram", bufs=1, space="DRAM") as dpool,
        ):
            # ---- constants ----
            ident_bf = cpool.tile([128, 128], bf16)
            make_identity(nc, ident_bf)
            ident_f32 = cpool.tile([128, 128], f32)
            make_identity(nc, ident_f32)

            bias_sb = cpool.tile([128, 4], f32)
            for t in range(4):
                nc.sync.dma_start(bias_sb[:, t:t + 1], bout[128 * t:128 * (t + 1), :])

            wa_sb = [cpool.tile([128, 384], bf16, name=f"wa{f}") for f in range(16)]
            for f in range(16):
                nc.sync.dma_start(wa_sb[f][:], waT[128 * f:128 * (f + 1), :])
            wp_sb = [cpool.tile([128, 384], bf16, name=f"wp{f}") for f in range(8)]
            wk_sb = [cpool.tile([128, 384], bf16, name=f"wk{f}") for f in range(8)]
            for f in range(8):
                nc.sync.dma_start(wp_sb[f][:], wpT[128 * f:128 * (f + 1), :])
                nc.sync.dma_start(wk_sb[f][:], wkT[128 * f:128 * (f + 1), :])
            wo_sb = [cpool.tile([128, DOUT], bf16, name=f"wo{f}") for f in range(4)]
            for f in range(4):
                nc.sync.dma_start(wo_sb[f][:], woutT[128 * f:128 * (f + 1), :])

            # ---- persistent activations ----
            # qT/kT per branch: [128, N]; heads live at 32-aligned partition bases
            qTa = cpool.tile([128, N], bf16)
            kTa = cpool.tile([128, N], bf16)
            qTp = cpool.tile([128, N], bf16)
            kTp = cpool.tile([128, N], bf16)
            qTk = cpool.tile([128, N], bf16)
            kTk = cpool.tile([128, N], bf16)
            # V^T combined: head h at rows 64*(h%2)+[va(32)|vp(16)|vk(16)] of tile h//2
            comb = [cpool.tile([128, N], bf16, name=f"comb{i}") for i in range(2)]
            # V_aug per head: 16 chunks of [128, 128] side by side: cols
            # 0:64 = v, col 64 = ones, 65:128 = zeros (M=128 keeps the PE
            # array fully configured so HAM doesn't throttle the clock)
            vaug = [cpool.tile([128, 128 * NJ], bf16, name=f"vaug{h}") for h in range(H_LOC)]
            # normalized attention output accumulator, [token, dv] layout
            oacc = [[cpool.tile([128, DV], f32, name=f"oacc{h}_{s}") for s in range(N // 128)]
                    for h in range(H_LOC)]
            # final transposed attention output (this core's heads)
            otc = [cpool.tile([128, N], bf16, name=f"otc{i}") for i in range(2)]

            cc_in = dpool.tile([2 * 128, N], bf16)
            cc_out = dpool.tile([4 * 128, N], bf16)

            for h in range(H_LOC):
                nc.gpsimd.memset(vaug[h][:], 0.0)
                for j in range(NJ):
                    nc.gpsimd.memset(vaug[h][:, 128 * j + 64:128 * j + 65], 1.0)

            _mctx = tc.tile_pool(name="mask", bufs=1)
            mpool = _mctx.__enter__()
            m_sb = [mpool.tile([128, N], bf16, name=f"m{j}") for j in range(NJ)]

            # =================== QKV projection ===================
            with (
                tc.tile_pool(name="xs", bufs=8) as xpool,
                tc.tile_pool(name="qkv_ps", bufs=5, space="PSUM") as qkv_ps,
                tc.tile_pool(name="vtr_ps", bufs=2, space="PSUM") as vtr_ps,
            ):
                # branch spec: (x row offset, n f-chunks, weights)
                # p and k share a v accumulator: vp at psum rows 32h+0:16,
                # vk at 32h+16:32 (host-padded weight columns) so every
                # PSUM read is 32-partition aligned.
                branches = [
                    (0, 16, wa_sb),
                    (DA, 8, wp_sb),
                    (DA + DP, 8, wk_sb),
                ]
                # pass 1: all v projections (attention needs the full
                # concatenated V, so finish it first).  Token-pair x tiles
                # ([128, 1024]) halve the DMA count.
                for tp2 in range(2):
                    t0 = 2 * IB * tp2
                    ps_va = [qkv_ps.tile([128, IB], f32, tag="qkv", name=f"psva{u}")
                             for u in range(2)]
                    ps_vpk = [qkv_ps.tile([128, IB], f32, tag="qkv", name=f"psvpk{u}")
                              for u in range(2)]
                    for bi, (fofs, nf, wsb) in enumerate(branches):
                        ps_v = ps_va if bi == 0 else ps_vpk
                        for f in range(nf):
                            xt = xpool.tile([128, 2 * IB], bf16, tag="x")
                            nc.sync.dma_start(
                                xt[:], xT[fofs + 128 * f:fofs + 128 * (f + 1),
                                          t0:t0 + 2 * IB])
                            vst = (f == 0) and bi != 2
                            vsp = (f == nf - 1) and bi != 1
                            for u in range(2):
                                nc.tensor.matmul(ps_v[u][:], wsb[f][:, 256:384],
                                                 xt[:, IB * u:IB * (u + 1)],
                                                 start=vst, stop=vsp)
                    for u in range(2):
                        tsl = slice(t0 + IB * u, t0 + IB * (u + 1))
                        for h in range(H_LOC):
                            nc.vector.tensor_copy(
                                comb[h // 2][64 * (h % 2):64 * (h % 2) + da, tsl],
                                ps_va[u][da * h:da * (h + 1), :])
                            nc.vector.tensor_copy(
                                comb[h // 2][64 * (h % 2) + da:64 * (h % 2) + 64, tsl],
                                ps_vpk[u][32 * h:32 * (h + 1), :])

                # V_aug: transpose comb chunks
                for j in range(NJ):
                    jsl = slice(128 * j, 128 * (j + 1))
                    for c in range(2):
                        tp = vtr_ps.tile([128, 128], bf16, tag="vtr")
                        nc.tensor.transpose(tp[:], comb[c][:, jsl], ident_bf[:])
                        nc.vector.tensor_copy(vaug[2 * c][:, 128 * j:128 * j + 64], tp[:, 0:64])
                        nc.vector.tensor_copy(vaug[2 * c + 1][:, 128 * j:128 * j + 64], tp[:, 64:128])

                # pass 2: q/k per branch — branch a first so its attention
                # can start while p/k still project
                for bi, (fofs, nf, wsb) in enumerate(branches):
                    if bi == 1:
                        # mask tiles: issued once branch-a q/k DMAs are in the
                        # queues; they land before attention consumes them
                        for j in range(NJ):
                            nc.sync.dma_start(m_sb[j][:],
                                              maskT[128 * j:128 * (j + 1), :])
                    for tp2 in range(2):
                        t0 = 2 * IB * tp2
                        ps_q = [qkv_ps.tile([128, IB], f32, tag="qkv", name=f"psq{u}")
                                for u in range(2)]
                        ps_k = [qkv_ps.tile([128, IB], f32, tag="qkv", name=f"psk{u}")
                                for u in range(2)]
                        for f in range(nf):
                            xt = xpool.tile([128, 2 * IB], bf16, tag="x")
                            nc.sync.dma_start(
                                xt[:], xT[fofs + 128 * f:fofs + 128 * (f + 1),
                                          t0:t0 + 2 * IB])
                            st, sp = (f == 0), (f == nf - 1)
                            w = wsb[f]
                            for u in range(2):
                                nc.tensor.matmul(ps_q[u][:], w[:, 0:128],
                                                 xt[:, IB * u:IB * (u + 1)],
                                                 start=st, stop=sp)
                                nc.tensor.matmul(ps_k[u][:], w[:, 128:256],
                                                 xt[:, IB * u:IB * (u + 1)],
                                                 start=st, stop=sp)
                        qT_t = (qTa, qTp, qTk)[bi]
                        kT_t = (kTa, kTp, kTk)[bi]
                        for u in range(2):
                            tsl = slice(t0 + IB * u, t0 + IB * (u + 1))
                            nc.vector.tensor_copy(qT_t[:, tsl], ps_q[u][:])
                            nc.vector.tensor_copy(kT_t[:, tsl], ps_k[u][:])

            # =================== attention ===================
            with (
                tc.tile_pool(name="s_ps", bufs=2, space="PSUM") as s_ps_pool,
                tc.tile_pool(name="o_ps", bufs=4, space="PSUM") as o_ps_pool,
                tc.tile_pool(name="ep", bufs=6) as epool,
                tc.tile_pool(name="pp", bufs=6) as ppool,
                tc.tile_pool(name="ob", bufs=3) as opool,
                tc.tile_pool(name="rr", bufs=4) as rpool,
            ):
                battn = [(qTa, kTa, da), (qTp, kTp, dp), (qTk, kTk, dk)]
                for bi, (qT_t, kT_t, d) in enumerate(battn):
                    for I in range(NI):
                        isl = slice(IB * I, IB * (I + 1))
                        # 2 halves of 2 heads each: each half has its own
                        # 2-bank S tile, so dots of one half overlap exp
                        # of the other; within a half the 2 row-disjoint
                        # dots are chained adjacent to run concurrently
                        o_ps_h = [o_ps_pool.tile([128, IB], f32, tag="o",
                                                 name=f"ops{h}")
                                  for h in range(H_LOC)]

                        def emit_av(jj, hf, pp_sb):
                            for hh in range(2):
                                h = 2 * hf + hh
                                nc.tensor.matmul(
                                    o_ps_h[h][:],
                                    vaug[h][:, 128 * jj:128 * (jj + 1)],
                                    pp_sb[:, IB * hh:IB * (hh + 1)],
                                    start=(jj == 0), stop=(jj == NJ - 1),
                                    skip_group_check=True)

                        # while DVE digests the previous block's epilogue
                        # (first ~5 j's), mask-multiplies go to gpsimd and
                        # their AVs are emitted two j's later so the slow
                        # gpsimd op can't head-of-line-block the PE stream
                        first_blk = (bi == 0 and I == 0)
                        av_backlog = []
                        for j in range(NJ):
                            for half in range(2):
                                s_ps = s_ps_pool.tile([128, 2 * IB], f32,
                                                      tag="s", name=f"sh{half}")
                                dots = []
                                for hh in range(2):
                                    h = 2 * half + hh
                                    pb = 32 * h
                                    mm = nc.tensor.matmul(
                                        s_ps[:, IB * hh:IB * (hh + 1)],
                                        kT_t[pb:pb + d, 128 * j:128 * (j + 1)],
                                        qT_t[pb:pb + d, isl],
                                        start=True, stop=True,
                                        tile_position=(pb, 0))
                                    if dots:
                                        add_dep_helper(mm.ins, dots[-1].ins,
                                                       sync=False,
                                                       reason="chain dots")
                                    dots.append(mm)
                                if half == 0:
                                    while av_backlog and av_backlog[0][0] <= j:
                                        _, jj, hf, pp_sb = av_backlog.pop(0)
                                        emit_av(jj, hf, pp_sb)
                                e_sb = epool.tile([128, 2 * IB], bf16, tag="e")
                                nc.scalar.activation(e_sb[:], s_ps[:], Exp)
                                p_sb = ppool.tile([128, 2 * IB], bf16, tag="p")
                                m_bc = m_sb[j][:, None, isl].broadcast_to(
                                    [128, 2, IB])
                                on_gp = False
                                teng = nc.vector
                                teng.tensor_tensor(
                                    p_sb[:].rearrange("p (g i) -> p g i", g=2),
                                    e_sb[:].rearrange("p (g i) -> p g i", g=2),
                                    m_bc, op=mult)
                                if on_gp:
                                    av_backlog.append((j + 2, j, half, p_sb))
                                else:
                                    emit_av(j, half, p_sb)
                        for _, jj, hf, pp_sb in av_backlog:
                            emit_av(jj, hf, pp_sb)
                        # epilogue: drain all four accumulators first so the
                        # next block's AVs get PSUM slots immediately, then
                        # normalize + accumulate
                        o_sbs = []
                        for h in range(H_LOC):
                            o_sb = opool.tile([65, IB], f32, tag="osb",
                                              name=f"osb{h}")
                            nc.vector.tensor_copy(o_sb[:], o_ps_h[h][0:65, :])
                            o_sbs.append(o_sb)
                        for h in range(H_LOC):
                            for s in range(IB // 128):
                                tp = o_ps_pool.tile([128, 65], f32, tag="o",
                                                    name="tps")
                                nc.tensor.transpose(
                                    tp[:], o_sbs[h][:, 128 * s:128 * (s + 1)],
                                    ident_f32[0:65, 0:65])
                                r_sb = rpool.tile([128, 1], f32, tag="r")
                                nc.vector.reciprocal(r_sb[:], tp[:, 64:65])
                                at = oacc[h][4 * I + s]
                                if bi == 0:
                                    nc.vector.tensor_scalar_mul(at[:], tp[:, 0:DV], r_sb[:])
                                else:
                                    nc.vector.scalar_tensor_tensor(
                                        at[:], tp[:, 0:DV], r_sb[:], at[:],
                                        op0=mult, op1=add)
                                if bi == 2:
                                    tp2 = o_ps_pool.tile([DV, 128], f32,
                                                         tag="o", name="t2")
                                    nc.tensor.transpose(tp2[:], at[:],
                                                        ident_f32[:])
                                    sl = 4 * I + s
                                    nc.vector.tensor_copy(
                                        otc[h // 2][64 * (h % 2):64 * (h % 2) + DV,
                                                    128 * sl:128 * (sl + 1)],
                                        tp2[:])

            _mctx.__exit__(None, None, None)

            # =================== gather + output projection ===================
            with (
                tc.tile_pool(name="fin", bufs=2) as fpool,
                tc.tile_pool(name="otf", bufs=1) as otfpool,
                tc.tile_pool(name="f_ps", bufs=2, space="PSUM") as f_ps_pool,
            ):
                for c in range(2):
                    nc.sync.dma_start(cc_in[128 * c:128 * (c + 1), :], otc[c][:])
                nc.gpsimd.collective_compute(
                    "AllGather",
                    mybir.AluOpType.bypass,
                    replica_groups=[[0, 1], [2, 3], [4, 5], [6, 7]],
                    ins=[cc_in.opt()],
                    outs=[cc_out.opt()],
                )
                otf = [otfpool.tile([128, N], bf16, name=f"otf{i}") for i in range(4)]
                for c in range(4):
                    nc.sync.dma_start(otf[c][:], cc_out[128 * c:128 * (c + 1), :])
                for ot in range(4):
                    for I2 in range(4):
                        i2sl = slice(512 * I2, 512 * (I2 + 1))
                        ps = f_ps_pool.tile([128, 512], f32, tag="f")
                        for ic in range(4):
                            nc.tensor.matmul(
                                ps[:], wo_sb[ic][:, 128 * ot:128 * (ot + 1)],
                                otf[ic][:, i2sl],
                                start=(ic == 0), stop=(ic == 3))
                        fin = fpool.tile([128, 512], f32, tag="fin")
                        nc.vector.tensor_scalar_add(fin[:], ps[:], bias_sb[:, ot:ot + 1])
                        nc.sync.dma_start(out[128 * ot:128 * (ot + 1), i2sl], fin[:])

    nc.compile()
    return nc


def _prep_core(c, x, W_a, W_p, W_k, W_out, b_out, mask):
    b = c // 2
    h0 = H_LOC * (c % 2)

    xT = np.ascontiguousarray(x[b].T).astype(BF16)
    maskT = np.ascontiguousarray(mask[b, 0].T).astype(BF16)

    qa = W_a[da * h0: da * (h0 + H_LOC), :] * (DA ** -0.5)
    ka = W_a[DA_H + da * h0: DA_H + da * (h0 + H_LOC), :]
    va = W_a[2 * DA_H + da * h0: 2 * DA_H + da * (h0 + H_LOC), :]
    waT = np.concatenate([qa.T, ka.T, va.T], axis=1).astype(BF16)

    def pk_branch(W, D, D_H, d, vcol_ofs):
        qpad = np.zeros((D, 128), np.float32)
        kpad = np.zeros((D, 128), np.float32)
        vpad = np.zeros((D, 128), np.float32)
        for h in range(H_LOC):
            qpad[:, 32 * h:32 * h + d] = W[d * (h0 + h): d * (h0 + h + 1), :].T * (D ** -0.5)
            kpad[:, 32 * h:32 * h + d] = W[D_H + d * (h0 + h): D_H + d * (h0 + h + 1), :].T
            vpad[:, 32 * h + vcol_ofs:32 * h + vcol_ofs + d] = \
                W[2 * D_H + d * (h0 + h): 2 * D_H + d * (h0 + h + 1), :].T
        return np.concatenate([qpad, kpad, vpad], axis=1).astype(BF16)

    wpT = pk_branch(W_p, DP, DP_H, dp, 0)
    wkT = pk_branch(W_k, DK, DK_H, dk, 16)

    woutT = np.ascontiguousarray((W_out / 3.0).T).astype(BF16)
    bout = np.ascontiguousarray(b_out.reshape(DOUT, 1)).astype(np.float32)

    return {
        "xT": np.ascontiguousarray(xT),
        "maskT": np.ascontiguousarray(maskT),
        "waT": np.ascontiguousarray(waT),
        "wpT": np.ascontiguousarray(wpT),
        "wkT": np.ascontiguousarray(wkT),
        "woutT": woutT,
        "bout": bout,
    }


def kernel(x, W_a, W_p, W_k, W_out, b_out, mask):
    from concourse.bass_utils import run_bass_kernel_spmd

    x = np.asarray(x, np.float32)
    W_a = np.asarray(W_a, np.float32)
    W_p = np.asarray(W_p, np.float32)
    W_k = np.asarray(W_k, np.float32)
    W_out = np.asarray(W_out, np.float32)
    b_out = np.asarray(b_out, np.float32)
    mask = np.asarray(mask)

    if "nc" not in _CACHE:
        _CACHE["nc"] = _build()
    nc = _CACHE["nc"]

    in_maps = [_prep_core(c, x, W_a, W_p, W_k, W_out, b_out, mask)
               for c in range(NCORES)]
    res = run_bass_kernel_spmd(nc, in_maps, core_ids=list(range(NCORES)))

    outs = []
    for b in range(B):
        outs.append(np.asarray(res.results[2 * b]["out"], np.float32).T)
    return np.stack(outs, axis=0)


# revision 52
# speedup vs baseline: 1.1539x; 1.0070x over previous
"""Distributed Trainium2 kernel for the 3-branch masked attention problem.

Sharding: 8 cores; core c handles batch b = c//2 and heads h0 = 4*(c%2) .. +4
(data + head parallel).  Each core computes QKV for its heads, the three
branch softmaxes and AV locally, then a pair-wise AllGather of the [256, 2048]
attention output (transposed) lets both cores of a batch apply the output
projection.  Host-side work is limited to sharding/layout (transposes, bf16
conversion, constant folding of d**-0.5 and the 1/3 branch average).
"""

import numpy as np
import ml_dtypes

BF16 = ml_dtypes.bfloat16

H = 8
DA, DP, DK = 2048, 1024, 1024
B, N = 4, 2048
DOUT = 512
H_LOC = 4           # heads per core
DA_H, DP_H, DK_H = DA // H, DP // H, DK // H      # 256, 128, 128
da, dp, dk = DA_H // H, DP_H // H, DK_H // H      # 32, 16, 16
DV = da + dp + dk                                 # 64
NCORES = 8

IB = 512            # query block (moving dim of dots / AV)
JB = 128            # key chunk (contract chunk of AV, M of dots)
NI = N // IB        # 4
NJ = N // JB        # 16

_CACHE = {}


def _build():
    import concourse.bass as bass
    import concourse.mybir as mybir
    import concourse.tile as tile
    from concourse import bacc
    from concourse.masks import make_identity
    from concourse.tile import add_dep_helper

    f32 = mybir.dt.float32
    bf16 = mybir.dt.bfloat16
    Exp = mybir.ActivationFunctionType.Exp
    mult = mybir.AluOpType.mult
    add = mybir.AluOpType.add

    nc = bacc.Bacc("TRN2", target_bir_lowering=False, debug=False,
                   enable_asserts=False, num_devices=NCORES)

    xT = nc.dram_tensor("xT", [DA + DP + DK, N], bf16, kind="ExternalInput")
    maskT = nc.dram_tensor("maskT", [N, N], bf16, kind="ExternalInput")
    waT = nc.dram_tensor("waT", [DA, 384], bf16, kind="ExternalInput")
    wpT = nc.dram_tensor("wpT", [DP, 384], bf16, kind="ExternalInput")
    wkT = nc.dram_tensor("wkT", [DK, 384], bf16, kind="ExternalInput")
    woutT = nc.dram_tensor("woutT", [DOUT, DOUT], bf16, kind="ExternalInput")
    bout = nc.dram_tensor("bout", [DOUT, 1], f32, kind="ExternalInput")
    out = nc.dram_tensor("out", [DOUT, N], bf16, kind="ExternalOutput")

    with tile.TileContext(nc) as tc:
        with (
            tc.tile_pool(name="const", bufs=1) as cpool,
            tc.tile_pool(name="dram", bufs=1, space="DRAM") as dpool,
        ):
            # ---- constants ----
            ident_bf = cpool.tile([128, 128], bf16)
            make_identity(nc, ident_bf)
            ident_f32 = cpool.tile([128, 128], f32)
            make_identity(nc, ident_f32)

            bias_sb = cpool.tile([128, 4], f32)
            for t in range(4):
                nc.sync.dma_start(bias_sb[:, t:t + 1], bout[128 * t:128 * (t + 1), :])

            wa_sb = [cpool.tile([128, 384], bf16, name=f"wa{f}") for f in range(16)]
            for f in range(16):
                nc.sync.dma_start(wa_sb[f][:], waT[128 * f:128 * (f + 1), :])
            wp_sb = [cpool.tile([128, 384], bf16, name=f"wp{f}") for f in range(8)]
            wk_sb = [cpool.tile([128, 384], bf16, name=f"wk{f}") for f in range(8)]
            for f in range(8):
                nc.sync.dma_start(wp_sb[f][:], wpT[128 * f:128 * (f + 1), :])
                nc.sync.dma_start(wk_sb[f][:], wkT[128 * f:128 * (f + 1), :])
            wo_sb = [cpool.tile([128, DOUT], bf16, name=f"wo{f}") for f in range(4)]
            for f in range(4):
                nc.sync.dma_start(wo_sb[f][:], woutT[128 * f:128 * (f + 1), :])

            # ---- persistent activations ----
            # qT/kT per branch: [128, N]; heads live at 32-aligned partition bases
            qTa = cpool.tile([128, N], bf16)
            kTa = cpool.tile([128, N], bf16)
            qTp = cpool.tile([128, N], bf16)
            kTp = cpool.tile([128, N], bf16)
            qTk = cpool.tile([128, N], bf16)
            kTk = cpool.tile([128, N], bf16)
            # V^T combined: head h at rows 64*(h%2)+[va(32)|vp(16)|vk(16)] of tile h//2
            comb = [cpool.tile([128, N], bf16, name=f"comb{i}") for i in range(2)]
            # V_aug per head: 16 chunks of [128, 128] side by side: cols
            # 0:64 = v, col 64 = ones, 65:128 = zeros (M=128 keeps the PE
            # array fully configured so HAM doesn't throttle the clock)
            vaug = [cpool.tile([128, 128 * NJ], bf16, name=f"vaug{h}") for h in range(H_LOC)]
            # normalized attention output accumulator, [token, dv] layout
            oacc = [[cpool.tile([128, DV], f32, name=f"oacc{h}_{s}") for s in range(N // 128)]
                    for h in range(H_LOC)]
            # final transposed attention output (this core's heads)
            otc = [cpool.tile([128, N], bf16, name=f"otc{i}") for i in range(2)]

            # per-token-half collective bounce buffers: the first half's
            # AllGather overlaps the tail of attention
            cc_in_h = [dpool.tile([2 * 128, N // 2], bf16, name=f"ccin{T}")
                       for T in range(2)]
            cc_out_h = [dpool.tile([4 * 128, N // 2], bf16, name=f"ccout{T}")
                        for T in range(2)]

            for h in range(H_LOC):
                nc.gpsimd.memset(vaug[h][:], 0.0)
                for j in range(NJ):
                    nc.gpsimd.memset(vaug[h][:, 128 * j + 64:128 * j + 65], 1.0)

            _mctx = tc.tile_pool(name="mask", bufs=1)
            mpool = _mctx.__enter__()
            m_sb = [mpool.tile([128, N], bf16, name=f"m{j}") for j in range(NJ)]

            # =================== QKV projection ===================
            with (
                tc.tile_pool(name="xs", bufs=8) as xpool,
                tc.tile_pool(name="qkv_ps", bufs=5, space="PSUM") as qkv_ps,
                tc.tile_pool(name="vtr_ps", bufs=2, space="PSUM") as vtr_ps,
            ):
                # branch spec: (x row offset, n f-chunks, weights)
                # p and k share a v accumulator: vp at psum rows 32h+0:16,
                # vk at 32h+16:32 (host-padded weight columns) so every
                # PSUM read is 32-partition aligned.
                branches = [
                    (0, 16, wa_sb),
                    (DA, 8, wp_sb),
                    (DA + DP, 8, wk_sb),
                ]
                # pass 1: all v projections (attention needs the full
                # concatenated V, so finish it first).  Token-pair x tiles
                # ([128, 1024]) halve the DMA count.
                for tp2 in range(2):
                    t0 = 2 * IB * tp2
                    ps_va = [qkv_ps.tile([128, IB], f32, tag="qkv", name=f"psva{u}")
                             for u in range(2)]
                    ps_vpk = [qkv_ps.tile([128, IB], f32, tag="qkv", name=f"psvpk{u}")
                              for u in range(2)]
                    for bi, (fofs, nf, wsb) in enumerate(branches):
                        ps_v = ps_va if bi == 0 else ps_vpk
                        for f in range(nf):
                            xt = xpool.tile([128, 2 * IB], bf16, tag="x")
                            nc.sync.dma_start(
                                xt[:], xT[fofs + 128 * f:fofs + 128 * (f + 1),
                                          t0:t0 + 2 * IB])
                            vst = (f == 0) and bi != 2
                            vsp = (f == nf - 1) and bi != 1
                            for u in range(2):
                                nc.tensor.matmul(ps_v[u][:], wsb[f][:, 256:384],
                                                 xt[:, IB * u:IB * (u + 1)],
                                                 start=vst, stop=vsp)
                    for u in range(2):
                        tsl = slice(t0 + IB * u, t0 + IB * (u + 1))
                        for h in range(H_LOC):
                            nc.vector.tensor_copy(
                                comb[h // 2][64 * (h % 2):64 * (h % 2) + da, tsl],
                                ps_va[u][da * h:da * (h + 1), :])
                            nc.vector.tensor_copy(
                                comb[h // 2][64 * (h % 2) + da:64 * (h % 2) + 64, tsl],
                                ps_vpk[u][32 * h:32 * (h + 1), :])

                # V_aug: transpose comb chunks
                for j in range(NJ):
                    jsl = slice(128 * j, 128 * (j + 1))
                    for c in range(2):
                        tp = vtr_ps.tile([128, 128], bf16, tag="vtr")
                        nc.tensor.transpose(tp[:], comb[c][:, jsl], ident_bf[:])
                        nc.vector.tensor_copy(vaug[2 * c][:, 128 * j:128 * j + 64], tp[:, 0:64])
                        nc.vector.tensor_copy(vaug[2 * c + 1][:, 128 * j:128 * j + 64], tp[:, 64:128])

                # pass 2: q/k per branch — branch a first so its attention
                # can start while p/k still project
                for bi, (fofs, nf, wsb) in enumerate(branches):
                    if bi == 1:
                        # mask tiles: issued once branch-a q/k DMAs are in the
                        # queues; they land before attention consumes them
                        for j in range(NJ):
                            nc.sync.dma_start(m_sb[j][:],
                                              maskT[128 * j:128 * (j + 1), :])
                    for tp2 in range(2):
                        t0 = 2 * IB * tp2
                        ps_q = [qkv_ps.tile([128, IB], f32, tag="qkv", name=f"psq{u}")
                                for u in range(2)]
                        ps_k = [qkv_ps.tile([128, IB], f32, tag="qkv", name=f"psk{u}")
                                for u in range(2)]
                        for f in range(nf):
                            xt = xpool.tile([128, 2 * IB], bf16, tag="x")
                            nc.sync.dma_start(
                                xt[:], xT[fofs + 128 * f:fofs + 128 * (f + 1),
                                          t0:t0 + 2 * IB])
                            st, sp = (f == 0), (f == nf - 1)
                            w = wsb[f]
                            for u in range(2):
                                nc.tensor.matmul(ps_q[u][:], w[:, 0:128],
                                                 xt[:, IB * u:IB * (u + 1)],
                                                 start=st, stop=sp)
                                nc.tensor.matmul(ps_k[u][:], w[:, 128:256],
                                                 xt[:, IB * u:IB * (u + 1)],
                                                 start=st, stop=sp)
                        qT_t = (qTa, qTp, qTk)[bi]
                        kT_t = (kTa, kTp, kTk)[bi]
                        for u in range(2):
                            tsl = slice(t0 + IB * u, t0 + IB * (u + 1))
                            nc.vector.tensor_copy(qT_t[:, tsl], ps_q[u][:])
                            nc.vector.tensor_copy(kT_t[:, tsl], ps_k[u][:])

            # =================== attention ===================
            with (
                tc.tile_pool(name="otf", bufs=1) as otfpool,
                tc.tile_pool(name="s_ps", bufs=2, space="PSUM") as s_ps_pool,
                tc.tile_pool(name="o_ps", bufs=4, space="PSUM") as o_ps_pool,
                tc.tile_pool(name="ep", bufs=5) as epool,
                tc.tile_pool(name="pp", bufs=5) as ppool,
                tc.tile_pool(name="ob", bufs=3) as opool,
                tc.tile_pool(name="rr", bufs=4) as rpool,
            ):
                otf_h = [[otfpool.tile([128, N // 2], bf16, name=f"otf{T}_{c}")
                          for c in range(4)] for T in range(2)]
                battn = [(qTa, kTa, da), (qTp, kTp, dp), (qTk, kTk, dk)]
                for bi, (qT_t, kT_t, d) in enumerate(battn):
                    for I in range(NI):
                        isl = slice(IB * I, IB * (I + 1))
                        # 2 halves of 2 heads each: each half has its own
                        # 2-bank S tile, so dots of one half overlap exp
                        # of the other; within a half the 2 row-disjoint
                        # dots are chained adjacent to run concurrently
                        o_ps_h = [o_ps_pool.tile([128, IB], f32, tag="o",
                                                 name=f"ops{h}")
                                  for h in range(H_LOC)]

                        def emit_av(jj, hf, pp_sb):
                            for hh in range(2):
                                h = 2 * hf + hh
                                nc.tensor.matmul(
                                    o_ps_h[h][:],
                                    vaug[h][:, 128 * jj:128 * (jj + 1)],
                                    pp_sb[:, IB * hh:IB * (hh + 1)],
                                    start=(jj == 0), stop=(jj == NJ - 1),
                                    skip_group_check=True)

                        # while DVE digests the previous block's epilogue
                        # (first ~5 j's), mask-multiplies go to gpsimd and
                        # their AVs are emitted two j's later so the slow
                        # gpsimd op can't head-of-line-block the PE stream
                        first_blk = (bi == 0 and I == 0)
                        av_backlog = []
                        for j in range(NJ):
                            for half in range(2):
                                s_ps = s_ps_pool.tile([128, 2 * IB], f32,
                                                      tag="s", name=f"sh{half}")
                                dots = []
                                for hh in range(2):
                                    h = 2 * half + hh
                                    pb = 32 * h
                                    mm = nc.tensor.matmul(
                                        s_ps[:, IB * hh:IB * (hh + 1)],
                                        kT_t[pb:pb + d, 128 * j:128 * (j + 1)],
                                        qT_t[pb:pb + d, isl],
                                        start=True, stop=True,
                                        tile_position=(pb, 0))
                                    if dots:
                                        add_dep_helper(mm.ins, dots[-1].ins,
                                                       sync=False,
                                                       reason="chain dots")
                                    dots.append(mm)
                                if half == 0:
                                    while av_backlog and av_backlog[0][0] <= j:
                                        _, jj, hf, pp_sb = av_backlog.pop(0)
                                        emit_av(jj, hf, pp_sb)
                                e_sb = epool.tile([128, 2 * IB], bf16, tag="e")
                                nc.scalar.activation(e_sb[:], s_ps[:], Exp)
                                p_sb = ppool.tile([128, 2 * IB], bf16, tag="p")
                                m_bc = m_sb[j][:, None, isl].broadcast_to(
                                    [128, 2, IB])
                                on_gp = False
                                teng = nc.vector
                                teng.tensor_tensor(
                                    p_sb[:].rearrange("p (g i) -> p g i", g=2),
                                    e_sb[:].rearrange("p (g i) -> p g i", g=2),
                                    m_bc, op=mult)
                                if on_gp:
                                    av_backlog.append((j + 2, j, half, p_sb))
                                else:
                                    emit_av(j, half, p_sb)
                        for _, jj, hf, pp_sb in av_backlog:
                            emit_av(jj, hf, pp_sb)
                        # epilogue: drain all four accumulators first so the
                        # next block's AVs get PSUM slots immediately, then
                        # normalize + accumulate
                        o_sbs = []
                        for h in range(H_LOC):
                            o_sb = opool.tile([65, IB], f32, tag="osb",
                                              name=f"osb{h}")
                            nc.vector.tensor_copy(o_sb[:], o_ps_h[h][0:65, :])
                            o_sbs.append(o_sb)
                        for h in range(H_LOC):
                            for s in range(IB // 128):
                                tp = o_ps_pool.tile([128, 65], f32, tag="o",
                                                    name="tps")
                                nc.tensor.transpose(
                                    tp[:], o_sbs[h][:, 128 * s:128 * (s + 1)],
                                    ident_f32[0:65, 0:65])
                                r_sb = rpool.tile([128, 1], f32, tag="r")
                                nc.vector.reciprocal(r_sb[:], tp[:, 64:65])
                                at = oacc[h][4 * I + s]
                                if bi == 0:
                                    nc.vector.tensor_scalar_mul(at[:], tp[:, 0:DV], r_sb[:])
                                else:
                                    nc.vector.scalar_tensor_tensor(
                                        at[:], tp[:, 0:DV], r_sb[:], at[:],
                                        op0=mult, op1=add)
                                if bi == 2:
                                    tp2 = o_ps_pool.tile([DV, 128], f32,
                                                         tag="o", name="t2")
                                    nc.tensor.transpose(tp2[:], at[:],
                                                        ident_f32[:])
                                    sl = 4 * I + s
                                    nc.vector.tensor_copy(
                                        otc[h // 2][64 * (h % 2):64 * (h % 2) + DV,
                                                    128 * sl:128 * (sl + 1)],
                                        tp2[:])
                        if bi == 2 and I in (1, 3):
                            # token half T of the attention output is final:
                            # gather it across the core pair now so the
                            # collective+DMA latency hides under attention
                            T = I // 2
                            hsl = slice(1024 * T, 1024 * (T + 1))
                            for c in range(2):
                                nc.sync.dma_start(
                                    cc_in_h[T][128 * c:128 * (c + 1), :],
                                    otc[c][:, hsl])
                            nc.gpsimd.collective_compute(
                                "AllGather",
                                mybir.AluOpType.bypass,
                                replica_groups=[[0, 1], [2, 3], [4, 5], [6, 7]],
                                ins=[cc_in_h[T].opt()],
                                outs=[cc_out_h[T].opt()],
                            )
                            for c in range(4):
                                nc.sync.dma_start(
                                    otf_h[T][c][:],
                                    cc_out_h[T][128 * c:128 * (c + 1), :])

                # =================== output projection ===================
                for T in range(2):
                    for ot in range(4):
                        for i2 in range(2):
                            i2sl = slice(512 * i2, 512 * (i2 + 1))
                            ps = o_ps_pool.tile([128, 512], f32, tag="o",
                                                name="fps")
                            for ic in range(4):
                                nc.tensor.matmul(
                                    ps[:], wo_sb[ic][:, 128 * ot:128 * (ot + 1)],
                                    otf_h[T][ic][:, i2sl],
                                    start=(ic == 0), stop=(ic == 3))
                            fin = epool.tile([128, 512], bf16, tag="e",
                                             name="fin")
                            nc.vector.tensor_scalar_add(fin[:], ps[:],
                                                        bias_sb[:, ot:ot + 1])
                            nc.sync.dma_start(
                                out[128 * ot:128 * (ot + 1),
                                    1024 * T + 512 * i2:1024 * T + 512 * (i2 + 1)],
                                fin[:])

            _mctx.__exit__(None, None, None)

    nc.compile()
    return nc


def _prep_core(c, x, W_a, W_p, W_k, W_out, b_out, mask):
    b = c // 2
    h0 = H_LOC * (c % 2)

    xT = np.ascontiguousarray(x[b].T).astype(BF16)
    maskT = np.ascontiguousarray(mask[b, 0].T).astype(BF16)

    qa = W_a[da * h0: da * (h0 + H_LOC), :] * (DA ** -0.5)
    ka = W_a[DA_H + da * h0: DA_H + da * (h0 + H_LOC), :]
    va = W_a[2 * DA_H + da * h0: 2 * DA_H + da * (h0 + H_LOC), :]
    waT = np.concatenate([qa.T, ka.T, va.T], axis=1).astype(BF16)

    def pk_branch(W, D, D_H, d, vcol_ofs):
        qpad = np.zeros((D, 128), np.float32)
        kpad = np.zeros((D, 128), np.float32)
        vpad = np.zeros((D, 128), np.float32)
        for h in range(H_LOC):
            qpad[:, 32 * h:32 * h + d] = W[d * (h0 + h): d * (h0 + h + 1), :].T * (D ** -0.5)
            kpad[:, 32 * h:32 * h + d] = W[D_H + d * (h0 + h): D_H + d * (h0 + h + 1), :].T
            vpad[:, 32 * h + vcol_ofs:32 * h + vcol_ofs + d] = \
                W[2 * D_H + d * (h0 + h): 2 * D_H + d * (h0 + h + 1), :].T
        return np.concatenate([qpad, kpad, vpad], axis=1).astype(BF16)

    wpT = pk_branch(W_p, DP, DP_H, dp, 0)
    wkT = pk_branch(W_k, DK, DK_H, dk, 16)

    woutT = np.ascontiguousarray((W_out / 3.0).T).astype(BF16)
    bout = np.ascontiguousarray(b_out.reshape(DOUT, 1)).astype(np.float32)

    return {
        "xT": np.ascontiguousarray(xT),
        "maskT": np.ascontiguousarray(maskT),
        "waT": np.ascontiguousarray(waT),
        "wpT": np.ascontiguousarray(wpT),
        "wkT": np.ascontiguousarray(wkT),
        "woutT": woutT,
        "bout": bout,
    }


def kernel(x, W_a, W_p, W_k, W_out, b_out, mask):
    from concourse.bass_utils import run_bass_kernel_spmd

    x = np.asarray(x, np.float32)
    W_a = np.asarray(W_a, np.float32)
    W_p = np.asarray(W_p, np.float32)
    W_k = np.asarray(W_k, np.float32)
    W_out = np.asarray(W_out, np.float32)
    b_out = np.asarray(b_out, np.float32)
    mask = np.asarray(mask)

    if "nc" not in _CACHE:
        _CACHE["nc"] = _build()
    nc = _CACHE["nc"]

    in_maps = [_prep_core(c, x, W_a, W_p, W_k, W_out, b_out, mask)
               for c in range(NCORES)]
    res = run_bass_kernel_spmd(nc, in_maps, core_ids=list(range(NCORES)))

    outs = []
    for b in range(B):
        outs.append(np.asarray(res.results[2 * b]["out"]).astype(np.float32).T)
    return np.stack(outs, axis=0)
